# revision 1
# baseline (speedup 1.0000x reference)
"""Trainium2 Bass kernel for nn_LocalOptLoss (batch 16384, data-parallel on 8 cores).

Layout: features on partitions, batch samples on the free dim. Each core gets
2048 samples, processed as 4 tiles of 512 (fp32 PSUM-bank free-dim limit).
Every Jacobian/Hessian term of the reference factors into W2 @ diag(d) @ W1
forms, so the whole per-sample computation is a chain of K<=32 matmuls plus
elementwise tanh/derivative ops.
"""
import sys

sys.path.insert(0, "/opt/trn_rl_repo")

from contextlib import ExitStack

import numpy as np
from ml_dtypes import bfloat16 as np_bf16

import concourse.bass as bass
import concourse.bacc as bacc
import concourse.tile as tile
from concourse import mybir
from concourse.bass_utils import run_bass_kernel_spmd
from concourse.tile_rust import add_dep_helper

N, NZ, OUT, H, B = 16, 16, 8, 32, 16384
R = 0.1
NCORES = 8
PER_CORE = B // NCORES          # 2048
F = 512                          # free-dim tile (samples per matmul)
NT = PER_CORE // F               # 4 tiles per core

F32 = mybir.dt.float32
BF16 = mybir.dt.bfloat16
AF = mybir.ActivationFunctionType
ALU = mybir.AluOpType

# Packed weight tile: every lhsT / bias lives in one (128, PCOLS) SBUF tile
# loaded by a single DMA. Entry: name -> (row0, nrows, col0, ncols).
_WLAYOUT = {}
_PC = 0


def _wadd(name, r0, nr, ncols):
    global _PC
    _WLAYOUT[name] = (r0, nr, _PC, ncols)
    _PC += ncols


_wadd("w_T1", 0, 16, 32)      # WT1.T
_wadd("w_T2", 0, 32, 16)      # WT2.T
_wadd("w_tau1", 0, 16, 32)    # Wtau1.T
_wadd("w_tau2", 0, 32, 16)    # Wtau2.T
_wadd("w_big", 0, 16, 128)    # [WT1; WP; 0; Wf1; Wh1].T
_wadd("w_h2_hi", 96, 32, 8)   # Wh2.T at base 96 (rhs th at base 96)
_wadd("w_psi1z", 0, 16, 32)
_wadd("w_psi1y", 0, 8, 32)
_wadd("w_psi2", 0, 32, 16)
_wadd("w_q", 0, 16, 64)       # [Wpsi1z; Wtau1].T -> [q3; q1]
_wadd("w_f1", 0, 16, 32)
_wadd("w_h1", 0, 16, 32)
_wadd("w_h2r", 0, 32, 8)      # (Wh2/R).T
_wadd("w_h2T", 0, 8, 32)      # Wh2
_wadd("w_h1T", 0, 32, 16)     # Wh1
_wadd("w_f2", 0, 32, 16)
_wadd("w_ntau2", 0, 32, 16)   # (-Wtau2).T
_wadd("w_tau2_hi", 32, 32, 16)  # Wtau2.T at base 32
_wadd("w_psi1y_hi", 32, 8, 32)  # Wpsi1y.T at base 32 (rhs yh)
_wadd("w_tau1_hi", 32, 16, 32)  # Wtau1.T at base 32 (rhs pp)
_wadd("ones16", 0, 16, 16)
_wadd("eye16", 0, 16, 16)
_wadd("onecol", 0, 16, 1)
_wadd("b_T1", 0, 32, 1)
_wadd("b_tau1", 0, 32, 1)
_wadd("b_big", 0, 128, 1)
_wadd("b_psi1", 0, 32, 1)
PCOLS = _PC


def build_nc():
    nc = bacc.Bacc("TRN2", target_bir_lowering=False, debug=False,
                   num_devices=NCORES)
    xs_d = nc.dram_tensor("xt", [N, PER_CORE], BF16, kind="ExternalInput")
    es_d = nc.dram_tensor("et", [NZ, PER_CORE], BF16, kind="ExternalInput")
    wp_d = nc.dram_tensor("wpack", [128, PCOLS], BF16, kind="ExternalInput")
    out_d = nc.dram_tensor("out", [1, NT], F32, kind="ExternalOutput")

    with tile.TileContext(nc) as tc, ExitStack() as ctx:
        wt = ctx.enter_context(tc.tile_pool(name="wt", bufs=1))
        acts = ctx.enter_context(tc.tile_pool(name="acts", bufs=3))
        ps = ctx.enter_context(tc.tile_pool(name="ps", bufs=6, space="PSUM"))
        psn = ctx.enter_context(tc.tile_pool(name="psn", bufs=2, space="PSUM"))

        wpack = wt.tile([128, PCOLS], BF16, tag="wpack", name="wpack")
        w = {k: wpack[r0:r0 + nr, c0:c0 + ncols]
             for k, (r0, nr, c0, ncols) in _WLAYOUT.items()}

        xs = wt.tile([N, PER_CORE], BF16, tag="xs")
        es = wt.tile([NZ, PER_CORE], BF16, tag="es")
        # Chain the three input DMAs so every downstream consumer needs at
        # most ONE DMA-queue wait (matmul/LDWEIGHTS carries a single wait).
        wdma = nc.gpsimd.dma_start(out=wpack, in_=wp_d.ap())
        xdma = nc.gpsimd.dma_start(out=xs, in_=xs_d.ap())
        edma = nc.gpsimd.dma_start(out=es, in_=es_d.ap())
        add_dep_helper(xdma.ins, wdma.ins, True, "serialize input DMAs")
        add_dep_helper(edma.ins, xdma.ins, True, "serialize input DMAs")

        accs = wt.tile([1, NT], F32, tag="accs")

        def mm(out, lhsT, rhs, start=True, stop=True):
            nc.tensor.matmul(out, lhsT, rhs, start=start, stop=stop)

        # Engine instructions carry at most ONE sync-wait. Warm each compute
        # engine's vector clock with tiny reads of the three DMA'd inputs so
        # no real instruction ever needs two DMA waits at once.
        warm_sb = wt.tile([1, 8], BF16, tag="warm_sb", name="warm_sb")
        for wi, src_ap in enumerate((wpack, xs, es)):
            p_warm = psn.tile([1, 16], F32, tag="psn", name=f"p_warm{wi}")
            mm(p_warm, w["onecol"], src_ap[0:16, 0:16])
            nc.scalar.activation(warm_sb[0:1, wi:wi + 1], src_ap[0:1, 0:1],
                                 AF.Copy)
            nc.vector.tensor_copy(warm_sb[0:1, 4 + wi:5 + wi],
                                  src_ap[0:1, 0:1])

        for i in range(NT):
            x_i = xs[:, i * F:(i + 1) * F]
            e_i = es[:, i * F:(i + 1) * F]

            # ---- forward chain ----
            p_pre1 = ps.tile([32, F], F32, tag="ps")
            mm(p_pre1, w["w_T1"], x_i)
            a1 = acts.tile([32, F], BF16, tag="a1")
            nc.scalar.activation(a1, p_pre1, AF.Tanh, bias=w["b_T1"])

            p_z = ps.tile([16, F], F32, tag="ps")
            mm(p_z, w["w_T2"], a1, start=True, stop=False)
            mm(p_z, w["eye16"], e_i, start=False, stop=True)
            z = acts.tile([16, F], BF16, tag="z")
            nc.scalar.activation(z, p_z, AF.Copy)

            p_pre2 = ps.tile([32, F], F32, tag="ps")
            mm(p_pre2, w["w_tau1"], z)
            a2 = acts.tile([32, F], BF16, tag="a2")
            nc.scalar.activation(a2, p_pre2, AF.Tanh, bias=w["b_tau1"])

            p_xh = ps.tile([16, F], F32, tag="ps")
            mm(p_xh, w["w_tau2"], a2)
            xh = acts.tile([16, F], BF16, tag="xh")
            nc.scalar.activation(xh, p_xh, AF.Copy)
            diff = acts.tile([16, F], BF16, tag="diff")
            nc.vector.tensor_sub(diff, x_i, p_xh)

            p_big = ps.tile([128, F], F32, tag="ps")
            mm(p_big, w["w_big"], xh)
            ta = acts.tile([128, F], BF16, tag="ta")   # [a3; s; pad; tf; th]
            nc.scalar.activation(ta, p_big, AF.Tanh, bias=w["b_big"])

            sqa = acts.tile([64, F], BF16, tag="sqa")
            nc.gpsimd.tensor_mul(sqa, ta[64:128], ta[64:128])
            D = acts.tile([128, F], BF16, tag="D")     # [df; dh; dpsi; dtau2]
            nc.vector.tensor_scalar(D[0:64], sqa, -1.0, 1.0, ALU.mult, ALU.add)

            p_Tx = ps.tile([16, F], F32, tag="ps")
            mm(p_Tx, w["w_T2"], ta[0:32])
            Tx = acts.tile([16, F], BF16, tag="Tx")
            nc.scalar.activation(Tx, p_Tx, AF.Copy)

            p_yh = ps.tile([8, F], F32, tag="ps")
            nc.tensor.matmul(p_yh, w["w_h2_hi"], ta[96:128], start=True,
                             stop=True, tile_position=(96, 0))
            yh = acts.tile([8, F], BF16, tag="yh")
            nc.scalar.activation(yh, p_yh, AF.Copy)

            p_ppsi = ps.tile([32, F], F32, tag="ps")
            mm(p_ppsi, w["w_psi1z"], Tx, start=True, stop=False)
            mm(p_ppsi, w["w_psi1y"], yh, start=False, stop=True)
            p_pt2 = ps.tile([32, F], F32, tag="ps")
            mm(p_pt2, w["w_tau1"], Tx)
            tb = acts.tile([64, F], BF16, tag="tb")    # [tp; t2]
            nc.scalar.activation(tb[0:32], p_ppsi, AF.Tanh, bias=w["b_psi1"])
            nc.scalar.activation(tb[32:64], p_pt2, AF.Tanh, bias=w["b_tau1"])

            sqb = acts.tile([64, F], BF16, tag="sqb")
            nc.gpsimd.tensor_mul(sqb, tb, tb)
            nc.vector.tensor_scalar(D[64:128], sqb, -1.0, 1.0, ALU.mult, ALU.add)

            p_phi = ps.tile([16, F], F32, tag="ps")
            mm(p_phi, w["w_psi2"], tb[0:32])
            phi = acts.tile([16, F], BF16, tag="phi")
            nc.scalar.activation(phi, p_phi, AF.Copy)

            # ---- JVP chains ----
            p_q = ps.tile([64, F], F32, tag="ps")     # [q3; q1]
            mm(p_q, w["w_q"], e_i)
            argA = acts.tile([64, F], BF16, tag="argA")  # [dpsi*q3; dtau2*q1]
            nc.vector.tensor_mul(argA, D[64:128], p_q)

            p_r = ps.tile([32, F], F32, tag="ps")
            mm(p_r, w["w_tau1"], phi)
            m2 = acts.tile([32, F], BF16, tag="m2")      # (-2 t2) * (dtau2 q1)
            nc.vector.scalar_tensor_tensor(m2, tb[32:64], -2.0, argA[32:64],
                                           ALU.mult, ALU.mult)
            argH = acts.tile([32, F], BF16, tag="argH")
            nc.vector.tensor_mul(argH, m2, p_r)

            p_up = ps.tile([48, F], F32, tag="ps")
            mm(p_up[0:16], w["w_tau2_hi"], argA[32:64])
            nc.tensor.matmul(p_up[32:48], w["w_psi2"], argA[0:32], start=True,
                             stop=True, tile_position=(0, 32))
            up = acts.tile([48, F], BF16, tag="up")
            nc.scalar.activation(up, p_up, AF.Copy)
            u = up[0:16]
            pp = up[32:48]

            p_q4 = ps.tile([32, F], F32, tag="ps")
            mm(p_q4, w["w_tau1_hi"], pp)
            argP = acts.tile([32, F], BF16, tag="argP")
            nc.vector.tensor_mul(argP, D[96:128], p_q4)

            p_q2 = ps.tile([32, F], F32, tag="ps")
            mm(p_q2, w["w_f1"], u)
            argF = acts.tile([32, F], BF16, tag="argF")
            nc.vector.tensor_mul(argF, D[0:32], p_q2)

            # ---- term1 chain ----
            p_hd = ps.tile([32, F], F32, tag="ps")
            mm(p_hd, w["w_h1"], diff)
            argh1 = acts.tile([32, F], BF16, tag="argh1")
            nc.vector.tensor_mul(argh1, D[32:64], p_hd)

            p_w = ps.tile([8, F], F32, tag="ps")
            mm(p_w, w["w_h2r"], argh1)
            ws = acts.tile([8, F], BF16, tag="ws")
            nc.scalar.activation(ws, p_w, AF.Copy)

            p_bk = ps.tile([32, F], F32, tag="ps")
            mm(p_bk, w["w_h2T"], ws)
            argh2 = acts.tile([32, F], BF16, tag="argh2")
            nc.vector.tensor_mul(argh2, D[32:64], p_bk)

            p_g = ps.tile([16, F], F32, tag="ps")
            mm(p_g, w["w_h1T"], argh2, start=True, stop=False)

            m = acts.tile([16, F], BF16, tag="m")        # s * g
            nc.vector.tensor_mul(m, ta[32:48], p_g)
            p_d = ps.tile([16, F], F32, tag="ps")
            mm(p_d, w["ones16"], m)                     # bcast(s.g)
            sm = acts.tile([16, F], BF16, tag="sm")      # s * bcast(s.g)
            nc.vector.tensor_mul(sm, ta[32:48], p_d)

            # accumulate g + Wf2 argF - Wtau2 argP - Wtau2 argH into p_g
            mm(p_g, w["w_f2"], argF, start=False, stop=False)
            mm(p_g, w["w_ntau2"], argP, start=False, stop=False)
            mm(p_g, w["w_ntau2"], argH, start=False, stop=True)

            v = acts.tile([16, F], BF16, tag="v")
            nc.vector.tensor_add(v, sm, p_g)
            vsq = acts.tile([16, F], BF16, tag="vsq")
            nc.gpsimd.tensor_mul(vsq, v, v)

            p_n = psn.tile([1, F], F32, tag="psn")
            mm(p_n, w["onecol"], vsq)
            nrm = acts.tile([1, F], F32, tag="nrm")
            nc.scalar.activation(nrm, p_n, AF.Sqrt,
                                 accum_out=accs[0:1, i:i + 1])

        nc.sync.dma_start(out=out_d.ap(), in_=accs)

    nc.compile()
    return nc


def _host_weights(Wf1, bf1, Wf2, Wh1, bh1, Wh2, WT1, bT1, WT2,
                  Wtau1, btau1, Wtau2, Wpsi1, bpsi1, Wpsi2, WP):
    f = np.float32
    T = lambda a: np.asarray(a, f).T
    Wpsi1z = np.asarray(Wpsi1, f)[:, :NZ]
    Wpsi1y = np.asarray(Wpsi1, f)[:, NZ:]
    vals = {
        "w_T1": T(WT1), "w_T2": T(WT2), "w_tau1": T(Wtau1), "w_tau2": T(Wtau2),
        "w_big": T(np.concatenate([WT1, WP, np.zeros((16, 16), f), Wf1, Wh1], 0)),
        "w_h2_hi": T(Wh2), "w_psi1z": T(Wpsi1z), "w_psi1y": T(Wpsi1y),
        "w_psi2": T(Wpsi2), "w_q": T(np.concatenate([Wpsi1z, Wtau1], 0)),
        "w_f1": T(Wf1), "w_h1": T(Wh1), "w_h2r": T(np.asarray(Wh2, f) / f(R)),
        "w_h2T": np.asarray(Wh2, f), "w_h1T": np.asarray(Wh1, f),
        "w_f2": T(Wf2), "w_ntau2": T(-np.asarray(Wtau2, f)),
        "w_tau2_hi": T(Wtau2),
        "w_psi1y_hi": T(Wpsi1y),
        "w_tau1_hi": T(Wtau1),
        "ones16": np.ones((16, 16), f), "eye16": np.eye(16, dtype=f),
        "onecol": np.ones((16, 1), f),
        "b_T1": np.asarray(bT1, f).reshape(-1, 1),
        "b_tau1": np.asarray(btau1, f).reshape(-1, 1),
        "b_big": np.concatenate([np.asarray(bT1, f), np.zeros(32, f),
                                 np.asarray(bf1, f),
                                 np.asarray(bh1, f)]).reshape(-1, 1),
        "b_psi1": np.asarray(bpsi1, f).reshape(-1, 1),
    }
    wpack = np.zeros((128, PCOLS), f)
    for k, (r0, nr, c0, ncols) in _WLAYOUT.items():
        v = vals[k]
        assert v.shape == (nr, ncols), (k, v.shape, (nr, ncols))
        wpack[r0:r0 + nr, c0:c0 + ncols] = v
    return {"wpack": wpack.astype(np_bf16)}


_CACHE = {}


def _get_nc():
    if "nc" not in _CACHE:
        _CACHE["nc"] = build_nc()
    return _CACHE["nc"]


def kernel(x_batch, e_batch, **wts):
    nc = _get_nc()
    wmap = _host_weights(**wts)
    xt = np.ascontiguousarray(np.asarray(x_batch, np.float32).T.astype(np_bf16))
    et = np.ascontiguousarray(np.asarray(e_batch, np.float32).T.astype(np_bf16))
    in_maps = []
    for c in range(NCORES):
        m = {"xt": np.ascontiguousarray(xt[:, c * PER_CORE:(c + 1) * PER_CORE]),
             "et": np.ascontiguousarray(et[:, c * PER_CORE:(c + 1) * PER_CORE])}
        m.update(wmap)
        in_maps.append(m)
    res = run_bass_kernel_spmd(nc, in_maps, core_ids=list(range(NCORES)))
    total = np.float64(0.0)
    for r in res.results:
        total += np.asarray(r["out"], np.float64).sum()
    return np.asarray(total / B, dtype=np.float32)


if __name__ == "__main__":
    rng = np.random.default_rng(0)
    # smoke test with random weights
    wts = {
        "Wf1": rng.normal(size=(H, N)) * .3, "bf1": rng.normal(size=(H,)) * .3,
        "Wf2": rng.normal(size=(N, H)) * .3,
        "Wh1": rng.normal(size=(H, N)) * .3, "bh1": rng.normal(size=(H,)) * .3,
        "Wh2": rng.normal(size=(OUT, H)) * .3,
        "WT1": rng.normal(size=(H, N)) * .3, "bT1": rng.normal(size=(H,)) * .3,
        "WT2": rng.normal(size=(NZ, H)) * .3,
        "Wtau1": rng.normal(size=(H, NZ)) * .3, "btau1": rng.normal(size=(H,)) * .3,
        "Wtau2": rng.normal(size=(N, H)) * .3,
        "Wpsi1": rng.normal(size=(H, NZ + OUT)) * .3, "bpsi1": rng.normal(size=(H,)) * .3,
        "Wpsi2": rng.normal(size=(NZ, H)) * .3,
        "WP": rng.normal(size=(N, N)) * .3,
    }
    x = rng.normal(size=(B, N)).astype(np.float32)
    e = (rng.normal(size=(B, NZ)) * 0.1).astype(np.float32)
    print(kernel(x, e, **{k: np.asarray(v, np.float32) for k, v in wts.items()}))



# revision 2
# speedup vs baseline: 2.8345x; 2.8345x over previous
"""Trainium2 Bass kernel for nn_LocalOptLoss (batch 16384, data-parallel on 8 cores).

v2: 4-lane block-diagonal packing. Each core gets 2048 samples laid out as
4 lanes x 512 columns; every per-sample matvec (K,M <= 32) becomes ONE
128x128-weight matmul over all four lanes, so the whole loss is ~25 matmul
instructions per core instead of ~110. Chained matvecs with no nonlinearity
between them are folded into host-precomputed weight products (e.g.
Wtau1 @ WT2), which removes every intermediate PSUM->SBUF copy. The final
per-sample vector v is DMA'd out raw; the host does norm + mean.
"""
import sys

sys.path.insert(0, "/opt/trn_rl_repo")

from contextlib import ExitStack

import numpy as np
from ml_dtypes import bfloat16 as np_bf16

import concourse.bass as bass
import concourse.bacc as bacc
import concourse.tile as tile
from concourse import mybir
from concourse.bass_utils import run_bass_kernel_spmd
from concourse.tile_rust import add_dep_helper

N, NZ, OUT, H, B = 16, 16, 8, 32, 16384
R = 0.1
NCORES = 8
PER_CORE = B // NCORES          # 2048
LANES = 4
COLS = PER_CORE // LANES        # 512 columns per lane
NT = 1                          # tiles (pipeline depth); F = COLS // NT
F = COLS // NT

F32 = mybir.dt.float32
BF16 = mybir.dt.bfloat16
AF = mybir.ActivationFunctionType
ALU = mybir.AluOpType

# ---- packed weight layout (block-diagonal lhsT tiles) ----
# entry: name -> (in_pitch, out_pitch, col0)  [K = 4*in_pitch, M = 4*out_pitch]
_WL = {}
_PC = [0, 0]   # col cursor for wcrit / wrest


def _wadd(pack, name, in_p, out_p):
    _WL[name] = (pack, in_p, out_p, _PC[pack])
    _PC[pack] += LANES * out_p


# critical pack: weights needed by the first chain links
_wadd(0, "w_pre1", 16, 32)      # WT1
_wadd(0, "w_pre2a", 32, 32)     # Wtau1 @ WT2   (also pt2)
_wadd(0, "w_tau1", 16, 32)      # Wtau1          (pre2b, q1)
# rest
_wadd(1, "w_a3", 32, 32)        # WT1 @ Wtau2
_wadd(1, "w_s", 32, 16)         # WP @ Wtau2
_wadd(1, "w_tf", 32, 32)        # Wf1 @ Wtau2    (tfpre, q2)
_wadd(1, "w_th", 32, 32)        # Wh1 @ Wtau2
_wadd(1, "w_hd1", 16, 32)       # Wh1
_wadd(1, "w_hd2n", 32, 32)      # -(Wh1 @ Wtau2)
_wadd(1, "w_ppsi1", 32, 32)     # Wpsi1z @ WT2
_wadd(1, "w_ppsi2", 32, 32)     # Wpsi1y @ Wh2
_wadd(1, "w_q3", 16, 32)        # Wpsi1z
_wadd(1, "w_r", 32, 32)         # Wtau1 @ Wpsi2  (r, q4)
_wadd(1, "w_bk", 32, 32)        # Wh2.T @ Wh2 / R
_wadd(1, "w_glin", 32, 16)      # Wh1 (as lhsT for Wh1.T @ .)
_wadd(1, "w_f2p", 32, 16)       # Wf2
_wadd(1, "w_tau2n", 32, 16)     # -Wtau2
_wadd(1, "w_ones", 16, 16)      # ones(16,16)
WCOLS0, WCOLS1 = _PC
BIAS_NAMES = ["bT1", "btau1", "bh1", "bpsi1", "bf1"]


def build_nc():
    nc = bacc.Bacc("TRN2", target_bir_lowering=False, debug=False,
                   num_devices=NCORES)
    xs_d = nc.dram_tensor("xt", [LANES * N, COLS], BF16, kind="ExternalInput")
    es_d = nc.dram_tensor("et", [LANES * NZ, COLS], BF16, kind="ExternalInput")
    w0_d = nc.dram_tensor("wcrit", [128, WCOLS0], BF16, kind="ExternalInput")
    w1_d = nc.dram_tensor("wrest", [128, WCOLS1], BF16, kind="ExternalInput")
    b_d = nc.dram_tensor("bpack", [128, len(BIAS_NAMES)], F32,
                         kind="ExternalInput")
    out_d = nc.dram_tensor("vout", [4 * N, COLS], BF16, kind="ExternalOutput")

    with tile.TileContext(nc) as tc, ExitStack() as ctx:
        wt = ctx.enter_context(tc.tile_pool(name="wt", bufs=1))
        acts = ctx.enter_context(tc.tile_pool(name="acts", bufs=1 if NT == 1 else 2))
        ps = ctx.enter_context(tc.tile_pool(name="ps", bufs=8, space="PSUM"))

        w0 = wt.tile([128, WCOLS0], BF16, tag="w0", name="w0")
        w1 = wt.tile([128, WCOLS1], BF16, tag="w1", name="w1")
        bp = wt.tile([128, len(BIAS_NAMES)], F32, tag="bp", name="bp")
        packs = {0: w0, 1: w1}
        w = {}
        for k, (p, in_p, out_p, c0) in _WL.items():
            w[k] = packs[p][0:LANES * in_p, c0:c0 + LANES * out_p]
        bias = {n: bp[:, i:i + 1] for i, n in enumerate(BIAS_NAMES)}

        xs = wt.tile([LANES * N, COLS], BF16, tag="xs")
        es = wt.tile([LANES * NZ, COLS], BF16, tag="es")
        vall = wt.tile([4 * N, COLS], BF16, tag="vall")

        # Chain input DMAs so each consumer carries at most one queue wait.
        d0 = nc.gpsimd.dma_start(out=xs, in_=xs_d.ap())
        d1 = nc.gpsimd.dma_start(out=bp, in_=b_d.ap())
        d2 = nc.gpsimd.dma_start(out=w0, in_=w0_d.ap())
        d3 = nc.gpsimd.dma_start(out=es, in_=es_d.ap())
        d4 = nc.gpsimd.dma_start(out=w1, in_=w1_d.ap())
        for a, b_ in ((d1, d0), (d2, d1), (d3, d2), (d4, d3)):
            add_dep_helper(a.ins, b_.ins, True, "serialize input DMAs")

        # Warm the Tanh activation table while DMAs are in flight.
        dummy = wt.tile([1, 1], BF16, tag="dummy", name="dummy")
        nc.vector.memset(dummy, 0.0)
        nc.scalar.activation(dummy, dummy, AF.Tanh)

        def mm(out, lhsT, rhs, start=True, stop=True):
            nc.tensor.matmul(out, lhsT, rhs, start=start, stop=stop)

        for t in range(NT):
            sl = slice(t * F, (t + 1) * F)
            x_t = xs[:, sl]
            e_t = es[:, sl]

            # ---- forward chain ----
            p_pre1 = ps.tile([128, F], F32, tag="ps")
            mm(p_pre1[0:128], w["w_pre1"], x_t)
            a1 = acts.tile([128, F], BF16, tag="a1")
            nc.scalar.activation(a1, p_pre1, AF.Tanh, bias=bias["bT1"])

            p_pre2 = ps.tile([128, F], F32, tag="ps")
            mm(p_pre2, w["w_pre2a"], a1, start=True, stop=False)
            mm(p_pre2, w["w_tau1"], e_t, start=False, stop=True)
            a2 = acts.tile([128, F], BF16, tag="a2")
            nc.scalar.activation(a2, p_pre2, AF.Tanh, bias=bias["btau1"])

            p_a3 = ps.tile([128, F], F32, tag="ps")
            mm(p_a3, w["w_a3"], a2)
            a3 = acts.tile([128, F], BF16, tag="a3")
            nc.scalar.activation(a3, p_a3, AF.Tanh, bias=bias["bT1"])

            p_s = ps.tile([128, F], F32, tag="ps")
            mm(p_s[0:64], w["w_s"], a2)
            s = acts.tile([64, F], BF16, tag="s")
            nc.scalar.activation(s, p_s[0:64], AF.Tanh)

            p_tf = ps.tile([128, F], F32, tag="ps")
            mm(p_tf, w["w_tf"], a2)
            tf = acts.tile([128, F], BF16, tag="tf")
            nc.scalar.activation(tf, p_tf, AF.Tanh, bias=bias["bf1"])
            sq_tf = acts.tile([128, F], BF16, tag="sq_tf")
            nc.scalar.activation(sq_tf, tf, AF.Square)

            p_th = ps.tile([128, F], F32, tag="ps")
            mm(p_th, w["w_th"], a2)
            th = acts.tile([128, F], BF16, tag="th")
            nc.scalar.activation(th, p_th, AF.Tanh, bias=bias["bh1"])
            sq_th = acts.tile([128, F], BF16, tag="sq_th")
            nc.gpsimd.tensor_mul(sq_th, th, th)

            p_hd = ps.tile([128, F], F32, tag="ps")
            mm(p_hd, w["w_hd1"], x_t, start=True, stop=False)
            mm(p_hd, w["w_hd2n"], a2, start=False, stop=True)
            argh1 = acts.tile([128, F], BF16, tag="argh1")
            nc.vector.scalar_tensor_tensor(argh1, sq_th, -1.0, p_hd,
                                           ALU.add, ALU.mult)

            p_ppsi = ps.tile([128, F], F32, tag="ps")
            mm(p_ppsi, w["w_ppsi1"], a3, start=True, stop=False)
            mm(p_ppsi, w["w_ppsi2"], th, start=False, stop=True)
            tp = acts.tile([128, F], BF16, tag="tp")
            nc.scalar.activation(tp, p_ppsi, AF.Tanh, bias=bias["bpsi1"])
            sq_tp = acts.tile([128, F], BF16, tag="sq_tp")
            nc.vector.tensor_mul(sq_tp, tp, tp)

            p_t2 = ps.tile([128, F], F32, tag="ps")
            mm(p_t2, w["w_pre2a"], a3)
            t2 = acts.tile([128, F], BF16, tag="t2")
            nc.scalar.activation(t2, p_t2, AF.Tanh, bias=bias["btau1"])
            sq_t2 = acts.tile([128, F], BF16, tag="sq_t2")
            nc.gpsimd.tensor_mul(sq_t2, t2, t2)

            # ---- JVP chains ----
            p_q1 = ps.tile([128, F], F32, tag="ps")
            mm(p_q1, w["w_tau1"], e_t)
            argA2n = acts.tile([128, F], BF16, tag="argA2n")
            nc.vector.scalar_tensor_tensor(argA2n, sq_t2, -1.0, p_q1,
                                           ALU.add, ALU.mult)

            p_q3 = ps.tile([128, F], F32, tag="ps")
            mm(p_q3, w["w_q3"], e_t)
            argA1 = acts.tile([128, F], BF16, tag="argA1")
            nc.vector.scalar_tensor_tensor(argA1, sq_tp, -1.0, p_q3,
                                           ALU.add, ALU.mult)

            p_r = ps.tile([128, F], F32, tag="ps")
            mm(p_r, w["w_r"], tp)
            mr = acts.tile([128, F], BF16, tag="mr")
            nc.vector.scalar_tensor_tensor(mr, t2, 2.0, p_r,
                                           ALU.mult, ALU.mult)
            argH = acts.tile([128, F], BF16, tag="argH")
            nc.gpsimd.tensor_mul(argH, mr, argA2n)

            p_q4 = ps.tile([128, F], F32, tag="ps")
            mm(p_q4, w["w_r"], argA1)
            argP = acts.tile([128, F], BF16, tag="argP")
            nc.vector.scalar_tensor_tensor(argP, sq_t2, -1.0, p_q4,
                                           ALU.add, ALU.mult)

            p_q2 = ps.tile([128, F], F32, tag="ps")
            mm(p_q2, w["w_tf"], argA2n)
            argF = acts.tile([128, F], BF16, tag="argF")
            nc.vector.scalar_tensor_tensor(argF, sq_tf, -1.0, p_q2,
                                           ALU.add, ALU.mult)

            # ---- term1 chain ----
            p_bk = ps.tile([128, F], F32, tag="ps")
            mm(p_bk, w["w_bk"], argh1)
            argh2 = acts.tile([128, F], BF16, tag="argh2")
            nc.vector.scalar_tensor_tensor(argh2, sq_th, -1.0, p_bk,
                                           ALU.add, ALU.mult)

            p_ga = ps.tile([128, F], F32, tag="ps")   # glin alone (bank A)
            mm(p_ga[0:64], w["w_glin"], argh2)
            m = acts.tile([64, F], BF16, tag="m")
            nc.vector.tensor_mul(m, s, p_ga[0:64])

            p_d = ps.tile([128, F], F32, tag="ps")
            mm(p_d[0:64], w["w_ones"], m)
            sm = acts.tile([64, F], BF16, tag="sm")
            nc.vector.tensor_mul(sm, s, p_d[0:64])

            # bank B: glin + Wf2 argF - Wtau2 argP - Wtau2 argH
            p_b = ps.tile([128, F], F32, tag="ps")
            mm(p_b[0:64], w["w_glin"], argh2, start=True, stop=False)
            mm(p_b[0:64], w["w_f2p"], argF, start=False, stop=False)
            mm(p_b[0:64], w["w_tau2n"], argP, start=False, stop=False)
            mm(p_b[0:64], w["w_tau2n"], argH, start=False, stop=True)

            nc.vector.tensor_add(vall[:, sl], sm, p_b[0:64])

        nc.sync.dma_start(out=out_d.ap(), in_=vall)

    nc.compile()
    return nc


def _host_weights(Wf1, bf1, Wf2, Wh1, bh1, Wh2, WT1, bT1, WT2,
                  Wtau1, btau1, Wtau2, Wpsi1, bpsi1, Wpsi2, WP):
    f = np.float64
    A = lambda a: np.asarray(a, f)
    Wf1, Wf2, Wh1, Wh2 = A(Wf1), A(Wf2), A(Wh1), A(Wh2)
    WT1, WT2, Wtau1, Wtau2 = A(WT1), A(WT2), A(Wtau1), A(Wtau2)
    Wpsi1, Wpsi2, WP = A(Wpsi1), A(Wpsi2), A(WP)
    Wpsi1z, Wpsi1y = Wpsi1[:, :NZ], Wpsi1[:, NZ:]

    vals = {
        "w_pre1": WT1, "w_pre2a": Wtau1 @ WT2, "w_tau1": Wtau1,
        "w_a3": WT1 @ Wtau2, "w_s": WP @ Wtau2, "w_tf": Wf1 @ Wtau2,
        "w_th": Wh1 @ Wtau2, "w_hd1": Wh1, "w_hd2n": -(Wh1 @ Wtau2),
        "w_ppsi1": Wpsi1z @ WT2, "w_ppsi2": Wpsi1y @ Wh2,
        "w_q3": Wpsi1z, "w_r": Wtau1 @ Wpsi2, "w_bk": Wh2.T @ Wh2 / R,
        "w_glin": Wh1.T, "w_f2p": Wf2, "w_tau2n": -Wtau2,
        "w_ones": np.ones((16, 16), f),
    }
    packs = {0: np.zeros((128, WCOLS0), f), 1: np.zeros((128, WCOLS1), f)}
    for k, (p, in_p, out_p, c0) in _WL.items():
        WT = vals[k].T          # (in_p, out_p) per-lane lhsT block
        assert WT.shape == (in_p, out_p), (k, WT.shape, (in_p, out_p))
        for L in range(LANES):
            packs[p][L * in_p:(L + 1) * in_p,
                     c0 + L * out_p:c0 + (L + 1) * out_p] = WT
    bvals = {"bT1": bT1, "btau1": btau1, "bh1": bh1, "bpsi1": bpsi1,
             "bf1": bf1}
    bpack = np.zeros((128, len(BIAS_NAMES)), np.float32)
    for i, n in enumerate(BIAS_NAMES):
        bpack[:, i] = np.tile(np.asarray(bvals[n], np.float32), LANES)
    return {"wcrit": packs[0].astype(np_bf16),
            "wrest": packs[1].astype(np_bf16), "bpack": bpack}


_CACHE = {}


def _get_nc():
    if "nc" not in _CACHE:
        _CACHE["nc"] = build_nc()
    return _CACHE["nc"]


def _in_maps(x_batch, e_batch, wts):
    wmap = _host_weights(**wts)

    def lanes(a, rows):
        # (PER_CORE, rows) -> (LANES*rows, COLS) with lane L = samples
        # [L*COLS, (L+1)*COLS)
        return np.ascontiguousarray(
            np.asarray(a, np.float32).reshape(LANES, COLS, rows)
            .transpose(0, 2, 1).reshape(LANES * rows, COLS).astype(np_bf16))

    in_maps = []
    for c in range(NCORES):
        cs = slice(c * PER_CORE, (c + 1) * PER_CORE)
        m = {"xt": lanes(x_batch[cs], N), "et": lanes(e_batch[cs], NZ)}
        m.update(wmap)
        in_maps.append(m)
    return in_maps


def _reduce(results):
    total = np.float64(0.0)
    for r in results:
        v = np.asarray(r["vout"], np.float64)      # (64, COLS)
        v = v.reshape(LANES, N, COLS)              # lane, feature, col
        total += np.sqrt((v * v).sum(axis=1)).sum()
    return np.asarray(total / B, dtype=np.float32)


def kernel(x_batch, e_batch, **wts):
    nc = _get_nc()
    in_maps = _in_maps(np.asarray(x_batch, np.float32),
                       np.asarray(e_batch, np.float32), wts)
    res = run_bass_kernel_spmd(nc, in_maps, core_ids=list(range(NCORES)))
    return _reduce(res.results)


if __name__ == "__main__":
    rng = np.random.default_rng(0)
    wts = {
        "Wf1": rng.normal(size=(H, N)) * .3, "bf1": rng.normal(size=(H,)) * .3,
        "Wf2": rng.normal(size=(N, H)) * .3,
        "Wh1": rng.normal(size=(H, N)) * .3, "bh1": rng.normal(size=(H,)) * .3,
        "Wh2": rng.normal(size=(OUT, H)) * .3,
        "WT1": rng.normal(size=(H, N)) * .3, "bT1": rng.normal(size=(H,)) * .3,
        "WT2": rng.normal(size=(NZ, H)) * .3,
        "Wtau1": rng.normal(size=(H, NZ)) * .3, "btau1": rng.normal(size=(H,)) * .3,
        "Wtau2": rng.normal(size=(N, H)) * .3,
        "Wpsi1": rng.normal(size=(H, NZ + OUT)) * .3, "bpsi1": rng.normal(size=(H,)) * .3,
        "Wpsi2": rng.normal(size=(NZ, H)) * .3,
        "WP": rng.normal(size=(N, N)) * .3,
    }
    x = rng.normal(size=(B, N)).astype(np.float32)
    e = (rng.normal(size=(B, NZ)) * 0.1).astype(np.float32)
    print(kernel(x, e, **{k: np.asarray(v, np.float32) for k, v in wts.items()}))


# revision 4
# speedup vs baseline: 3.8119x; 1.3448x over previous
"""Trainium2 Bass kernel for nn_LocalOptLoss (batch 16384, data-parallel on 8 cores).

v2: 4-lane block-diagonal packing. Each core gets 2048 samples laid out as
4 lanes x 512 columns; every per-sample matvec (K,M <= 32) becomes ONE
128x128-weight matmul over all four lanes, so the whole loss is ~25 matmul
instructions per core instead of ~110. Chained matvecs with no nonlinearity
between them are folded into host-precomputed weight products (e.g.
Wtau1 @ WT2), which removes every intermediate PSUM->SBUF copy. The final
per-sample vector v is DMA'd out raw; the host does norm + mean.
"""
import sys

sys.path.insert(0, "/opt/trn_rl_repo")

from contextlib import ExitStack

import numpy as np
from ml_dtypes import bfloat16 as np_bf16

import concourse.bass as bass
import concourse.bacc as bacc
import concourse.tile as tile
from concourse import mybir
from concourse.bass_utils import run_bass_kernel_spmd
from concourse.tile_rust import add_dep_helper

N, NZ, OUT, H, B = 16, 16, 8, 32, 16384
R = 0.1
NCORES = 8
PER_CORE = B // NCORES          # 2048
LANES = 4
COLS = PER_CORE // LANES        # 512 columns per lane
NT = 1                          # tiles (pipeline depth); F = COLS // NT
F = COLS // NT

F32 = mybir.dt.float32
BF16 = mybir.dt.bfloat16
AF = mybir.ActivationFunctionType
ALU = mybir.AluOpType

# ---- packed weight layout (block-diagonal lhsT tiles) ----
# entry: name -> (in_pitch, out_pitch, col0)  [K = 4*in_pitch, M = 4*out_pitch]
_WL = {}
_PC = [0, 0]   # col cursor for wcrit / wrest


def _wadd(pack, name, in_p, out_p):
    _WL[name] = (pack, in_p, out_p, _PC[pack])
    _PC[pack] += LANES * out_p


# critical pack: weights needed by the first chain links
_wadd(0, "w_pre1", 16, 32)      # WT1
_wadd(0, "w_pre2a", 32, 32)     # Wtau1 @ WT2   (also pt2)
_wadd(0, "w_tau1", 16, 32)      # Wtau1          (pre2b, q1)
# rest
_wadd(1, "w_a3", 32, 32)        # WT1 @ Wtau2
_wadd(1, "w_s", 32, 16)         # WP @ Wtau2
_wadd(1, "w_tf", 32, 32)        # Wf1 @ Wtau2    (tfpre, q2)
_wadd(1, "w_th", 32, 32)        # Wh1 @ Wtau2
_wadd(1, "w_hd1", 16, 32)       # Wh1
_wadd(1, "w_hd2n", 32, 32)      # -(Wh1 @ Wtau2)
_wadd(1, "w_ppsi1", 32, 32)     # Wpsi1z @ WT2
_wadd(1, "w_ppsi2", 32, 32)     # Wpsi1y @ Wh2
_wadd(1, "w_q3", 16, 32)        # Wpsi1z
_wadd(1, "w_r", 32, 32)         # Wtau1 @ Wpsi2  (r, q4)
_wadd(1, "w_bk", 32, 32)        # Wh2.T @ Wh2 / R
_wadd(1, "w_glin", 32, 16)      # Wh1 (as lhsT for Wh1.T @ .)
_wadd(1, "w_f2p", 32, 16)       # Wf2
_wadd(1, "w_tau2n", 32, 16)     # -Wtau2
_wadd(1, "w_ones", 16, 16)      # ones(16,16)
WCOLS0, WCOLS1 = _PC
BIAS_NAMES = ["bT1", "btau1", "bh1", "bpsi1", "bf1"]


def build_nc():
    nc = bacc.Bacc("TRN2", target_bir_lowering=False, debug=False,
                   num_devices=NCORES)
    xs_d = nc.dram_tensor("xt", [LANES * N, COLS], BF16, kind="ExternalInput")
    es_d = nc.dram_tensor("et", [LANES * NZ, COLS], BF16, kind="ExternalInput")
    w0_d = nc.dram_tensor("wcrit", [128, WCOLS0], BF16, kind="ExternalInput")
    w1_d = nc.dram_tensor("wrest", [128, WCOLS1], BF16, kind="ExternalInput")
    b_d = nc.dram_tensor("bpack", [128, len(BIAS_NAMES)], F32,
                         kind="ExternalInput")
    out_d = nc.dram_tensor("vout", [4 * N, COLS], BF16, kind="ExternalOutput")

    with tile.TileContext(nc) as tc, ExitStack() as ctx:
        wt = ctx.enter_context(tc.tile_pool(name="wt", bufs=1))
        acts = ctx.enter_context(tc.tile_pool(name="acts", bufs=1 if NT == 1 else 2))
        ps = ctx.enter_context(tc.tile_pool(name="ps", bufs=8, space="PSUM"))

        w0 = wt.tile([128, WCOLS0], BF16, tag="w0", name="w0")
        w1 = wt.tile([128, WCOLS1], BF16, tag="w1", name="w1")
        bp = wt.tile([128, len(BIAS_NAMES)], F32, tag="bp", name="bp")
        packs = {0: w0, 1: w1}
        w = {}
        for k, (p, in_p, out_p, c0) in _WL.items():
            w[k] = packs[p][0:LANES * in_p, c0:c0 + LANES * out_p]
        bias = {n: bp[:, i:i + 1] for i, n in enumerate(BIAS_NAMES)}

        xs = wt.tile([LANES * N, COLS], BF16, tag="xs")
        es = wt.tile([LANES * NZ, COLS], BF16, tag="es")
        vall = wt.tile([4 * N, COLS], BF16, tag="vall")

        # Issue input DMAs in parallel from otherwise-idle engine queues.
        nc.scalar.dma_start(out=bp, in_=b_d.ap())
        nc.sync.dma_start(out=w0, in_=w0_d.ap())
        nc.gpsimd.dma_start(out=xs, in_=xs_d.ap())
        nc.gpsimd.dma_start(out=es, in_=es_d.ap())
        nc.sync.dma_start(out=w1, in_=w1_d.ap())

        # Warm the Tanh activation table while DMAs are in flight; reading
        # bpack also advances the scalar engine's clock past the bpack DMA.
        dummy = wt.tile([1, 1], BF16, tag="dummy", name="dummy")
        nc.scalar.activation(dummy, bp[0:1, 0:1], AF.Tanh)

        def mm(out, lhsT, rhs, start=True, stop=True):
            nc.tensor.matmul(out, lhsT, rhs, start=start, stop=stop)

        for t in range(NT):
            sl = slice(t * F, (t + 1) * F)
            x_t = xs[:, sl]
            e_t = es[:, sl]

            # ---- forward chain ----
            p_pre1 = ps.tile([128, F], F32, tag="ps")
            mm(p_pre1[0:128], w["w_pre1"], x_t)
            a1 = acts.tile([128, F], BF16, tag="a1")
            nc.scalar.activation(a1, p_pre1, AF.Tanh, bias=bias["bT1"])

            p_pre2 = ps.tile([128, F], F32, tag="ps")
            mm(p_pre2, w["w_pre2a"], a1, start=True, stop=False)
            mm(p_pre2, w["w_tau1"], e_t, start=False, stop=True)
            a2 = acts.tile([128, F], BF16, tag="a2")
            nc.scalar.activation(a2, p_pre2, AF.Tanh, bias=bias["btau1"])

            p_a3 = ps.tile([128, F], F32, tag="ps")
            mm(p_a3, w["w_a3"], a2)
            a3 = acts.tile([128, F], BF16, tag="a3")
            nc.scalar.activation(a3, p_a3, AF.Tanh, bias=bias["bT1"])

            p_s = ps.tile([128, F], F32, tag="ps")
            mm(p_s[0:64], w["w_s"], a2)
            s = acts.tile([64, F], BF16, tag="s")
            nc.scalar.activation(s, p_s[0:64], AF.Tanh)

            p_tf = ps.tile([128, F], F32, tag="ps")
            mm(p_tf, w["w_tf"], a2)
            tf = acts.tile([128, F], BF16, tag="tf")
            nc.scalar.activation(tf, p_tf, AF.Tanh, bias=bias["bf1"])
            sq_tf = acts.tile([128, F], BF16, tag="sq_tf")
            nc.scalar.activation(sq_tf, tf, AF.Square)

            p_th = ps.tile([128, F], F32, tag="ps")
            mm(p_th, w["w_th"], a2)
            th = acts.tile([128, F], BF16, tag="th")
            nc.scalar.activation(th, p_th, AF.Tanh, bias=bias["bh1"])
            sq_th = acts.tile([128, F], BF16, tag="sq_th")
            nc.gpsimd.tensor_mul(sq_th, th, th)

            p_hd = ps.tile([128, F], F32, tag="ps")
            mm(p_hd, w["w_hd1"], x_t, start=True, stop=False)
            mm(p_hd, w["w_hd2n"], a2, start=False, stop=True)
            argh1 = acts.tile([128, F], BF16, tag="argh1")
            nc.vector.scalar_tensor_tensor(argh1, sq_th, -1.0, p_hd,
                                           ALU.add, ALU.mult)

            p_ppsi = ps.tile([128, F], F32, tag="ps")
            mm(p_ppsi, w["w_ppsi1"], a3, start=True, stop=False)
            mm(p_ppsi, w["w_ppsi2"], th, start=False, stop=True)
            tp = acts.tile([128, F], BF16, tag="tp")
            nc.scalar.activation(tp, p_ppsi, AF.Tanh, bias=bias["bpsi1"])
            sq_tp = acts.tile([128, F], BF16, tag="sq_tp")
            nc.vector.tensor_mul(sq_tp, tp, tp)

            p_t2 = ps.tile([128, F], F32, tag="ps")
            mm(p_t2, w["w_pre2a"], a3)
            t2 = acts.tile([128, F], BF16, tag="t2")
            nc.scalar.activation(t2, p_t2, AF.Tanh, bias=bias["btau1"])
            sq_t2 = acts.tile([128, F], BF16, tag="sq_t2")
            nc.gpsimd.tensor_mul(sq_t2, t2, t2)

            # ---- JVP chains ----
            p_q1 = ps.tile([128, F], F32, tag="ps")
            mm(p_q1, w["w_tau1"], e_t)
            argA2n = acts.tile([128, F], BF16, tag="argA2n")
            nc.vector.scalar_tensor_tensor(argA2n, sq_t2, -1.0, p_q1,
                                           ALU.add, ALU.mult)

            p_q3 = ps.tile([128, F], F32, tag="ps")
            mm(p_q3, w["w_q3"], e_t)
            argA1 = acts.tile([128, F], BF16, tag="argA1")
            nc.vector.scalar_tensor_tensor(argA1, sq_tp, -1.0, p_q3,
                                           ALU.add, ALU.mult)

            p_r = ps.tile([128, F], F32, tag="ps")
            mm(p_r, w["w_r"], tp)
            mr = acts.tile([128, F], BF16, tag="mr")
            nc.vector.scalar_tensor_tensor(mr, t2, 2.0, p_r,
                                           ALU.mult, ALU.mult)
            argH = acts.tile([128, F], BF16, tag="argH")
            nc.gpsimd.tensor_mul(argH, mr, argA2n)

            p_q4 = ps.tile([128, F], F32, tag="ps")
            mm(p_q4, w["w_r"], argA1)
            argP = acts.tile([128, F], BF16, tag="argP")
            nc.vector.scalar_tensor_tensor(argP, sq_t2, -1.0, p_q4,
                                           ALU.add, ALU.mult)

            p_q2 = ps.tile([128, F], F32, tag="ps")
            mm(p_q2, w["w_tf"], argA2n)
            argF = acts.tile([128, F], BF16, tag="argF")
            nc.vector.scalar_tensor_tensor(argF, sq_tf, -1.0, p_q2,
                                           ALU.add, ALU.mult)

            # ---- term1 chain ----
            p_bk = ps.tile([128, F], F32, tag="ps")
            mm(p_bk, w["w_bk"], argh1)
            argh2 = acts.tile([128, F], BF16, tag="argh2")
            nc.vector.scalar_tensor_tensor(argh2, sq_th, -1.0, p_bk,
                                           ALU.add, ALU.mult)

            p_ga = ps.tile([128, F], F32, tag="ps")   # glin alone (bank A)
            mm(p_ga[0:64], w["w_glin"], argh2)
            m = acts.tile([64, F], BF16, tag="m")
            nc.vector.tensor_mul(m, s, p_ga[0:64])

            p_d = ps.tile([128, F], F32, tag="ps")
            mm(p_d[0:64], w["w_ones"], m)
            sm = acts.tile([64, F], BF16, tag="sm")
            nc.vector.tensor_mul(sm, s, p_d[0:64])

            # bank B: glin + Wf2 argF - Wtau2 argP - Wtau2 argH
            p_b = ps.tile([128, F], F32, tag="ps")
            mm(p_b[0:64], w["w_glin"], argh2, start=True, stop=False)
            mm(p_b[0:64], w["w_f2p"], argF, start=False, stop=False)
            mm(p_b[0:64], w["w_tau2n"], argP, start=False, stop=False)
            mm(p_b[0:64], w["w_tau2n"], argH, start=False, stop=True)

            nc.vector.tensor_add(vall[:, sl], sm, p_b[0:64])

        nc.sync.dma_start(out=out_d.ap(), in_=vall)

    nc.compile()
    return nc


def _host_weights(Wf1, bf1, Wf2, Wh1, bh1, Wh2, WT1, bT1, WT2,
                  Wtau1, btau1, Wtau2, Wpsi1, bpsi1, Wpsi2, WP):
    f = np.float64
    A = lambda a: np.asarray(a, f)
    Wf1, Wf2, Wh1, Wh2 = A(Wf1), A(Wf2), A(Wh1), A(Wh2)
    WT1, WT2, Wtau1, Wtau2 = A(WT1), A(WT2), A(Wtau1), A(Wtau2)
    Wpsi1, Wpsi2, WP = A(Wpsi1), A(Wpsi2), A(WP)
    Wpsi1z, Wpsi1y = Wpsi1[:, :NZ], Wpsi1[:, NZ:]

    vals = {
        "w_pre1": WT1, "w_pre2a": Wtau1 @ WT2, "w_tau1": Wtau1,
        "w_a3": WT1 @ Wtau2, "w_s": WP @ Wtau2, "w_tf": Wf1 @ Wtau2,
        "w_th": Wh1 @ Wtau2, "w_hd1": Wh1, "w_hd2n": -(Wh1 @ Wtau2),
        "w_ppsi1": Wpsi1z @ WT2, "w_ppsi2": Wpsi1y @ Wh2,
        "w_q3": Wpsi1z, "w_r": Wtau1 @ Wpsi2, "w_bk": Wh2.T @ Wh2 / R,
        "w_glin": Wh1.T, "w_f2p": Wf2, "w_tau2n": -Wtau2,
        "w_ones": np.ones((16, 16), f),
    }
    packs = {0: np.zeros((128, WCOLS0), f), 1: np.zeros((128, WCOLS1), f)}
    for k, (p, in_p, out_p, c0) in _WL.items():
        WT = vals[k].T          # (in_p, out_p) per-lane lhsT block
        assert WT.shape == (in_p, out_p), (k, WT.shape, (in_p, out_p))
        for L in range(LANES):
            packs[p][L * in_p:(L + 1) * in_p,
                     c0 + L * out_p:c0 + (L + 1) * out_p] = WT
    bvals = {"bT1": bT1, "btau1": btau1, "bh1": bh1, "bpsi1": bpsi1,
             "bf1": bf1}
    bpack = np.zeros((128, len(BIAS_NAMES)), np.float32)
    for i, n in enumerate(BIAS_NAMES):
        bpack[:, i] = np.tile(np.asarray(bvals[n], np.float32), LANES)
    return {"wcrit": packs[0].astype(np_bf16),
            "wrest": packs[1].astype(np_bf16), "bpack": bpack}


_CACHE = {}


def _get_nc():
    if "nc" not in _CACHE:
        _CACHE["nc"] = build_nc()
    return _CACHE["nc"]


def _in_maps(x_batch, e_batch, wts):
    wmap = _host_weights(**wts)

    def lanes(a, rows):
        # (PER_CORE, rows) -> (LANES*rows, COLS) with lane L = samples
        # [L*COLS, (L+1)*COLS)
        return np.ascontiguousarray(
            np.asarray(a, np.float32).reshape(LANES, COLS, rows)
            .transpose(0, 2, 1).reshape(LANES * rows, COLS).astype(np_bf16))

    in_maps = []
    for c in range(NCORES):
        cs = slice(c * PER_CORE, (c + 1) * PER_CORE)
        m = {"xt": lanes(x_batch[cs], N), "et": lanes(e_batch[cs], NZ)}
        m.update(wmap)
        in_maps.append(m)
    return in_maps


def _reduce(results):
    total = np.float64(0.0)
    for r in results:
        v = np.asarray(r["vout"], np.float64)      # (64, COLS)
        v = v.reshape(LANES, N, COLS)              # lane, feature, col
        total += np.sqrt((v * v).sum(axis=1)).sum()
    return np.asarray(total / B, dtype=np.float32)


def kernel(x_batch, e_batch, **wts):
    nc = _get_nc()
    in_maps = _in_maps(np.asarray(x_batch, np.float32),
                       np.asarray(e_batch, np.float32), wts)
    res = run_bass_kernel_spmd(nc, in_maps, core_ids=list(range(NCORES)))
    return _reduce(res.results)


if __name__ == "__main__":
    rng = np.random.default_rng(0)
    wts = {
        "Wf1": rng.normal(size=(H, N)) * .3, "bf1": rng.normal(size=(H,)) * .3,
        "Wf2": rng.normal(size=(N, H)) * .3,
        "Wh1": rng.normal(size=(H, N)) * .3, "bh1": rng.normal(size=(H,)) * .3,
        "Wh2": rng.normal(size=(OUT, H)) * .3,
        "WT1": rng.normal(size=(H, N)) * .3, "bT1": rng.normal(size=(H,)) * .3,
        "WT2": rng.normal(size=(NZ, H)) * .3,
        "Wtau1": rng.normal(size=(H, NZ)) * .3, "btau1": rng.normal(size=(H,)) * .3,
        "Wtau2": rng.normal(size=(N, H)) * .3,
        "Wpsi1": rng.normal(size=(H, NZ + OUT)) * .3, "bpsi1": rng.normal(size=(H,)) * .3,
        "Wpsi2": rng.normal(size=(NZ, H)) * .3,
        "WP": rng.normal(size=(N, N)) * .3,
    }
    x = rng.normal(size=(B, N)).astype(np.float32)
    e = (rng.normal(size=(B, NZ)) * 0.1).astype(np.float32)
    print(kernel(x, e, **{k: np.asarray(v, np.float32) for k, v in wts.items()}))
